# revision 1
# baseline (speedup 1.0000x reference)
"""DenseCapsule dynamic-routing kernel for 8 Trainium2 NeuronCores.

Problem: x [256,1152,8] f32, weight [10,1152,16,8] f32 ->
  x_hat = einsum('oidc,bic->boid', weight, x)
  3 rounds of routing-by-agreement (softmax over o, squash over d)
  output [256, 10, 16] f32.

Strategy (batch-parallel over 8 cores, 32 samples/core):
  - i is split as i = k*8 + g  (k in [0,144), g in [0,8)).
  - Host builds block-diagonal weight stationaries WS[o,k] of shape [64,128]:
      WS[g*8+c, g*16+d] = W[o, k*8+g, d, c]
    and moving x blocks XM[g*8+c, k, b] = x[b, k*8+g, c], both fp16.
  - PE matmul per (o,k): out[(g,d), b] = x_hat[b,o,k*8+g,d] accumulated to
    PSUM fp32, copied to SBUF as XH fp16 with layout [p=(g,d), f=(o,k,b)].
  - Routing uses linearity of the logits: b_t = x_hat . (v_0+...+v_{t-1}),
    so no logits are stored across iterations; per iteration we recompute
    them from vsum with one elementwise pass + small PE reductions:
      z   = XH * vsum_rep          (DVE)
      bp  = S2^T @ z               (PE: sums d within each g, replicated)
      e   = exp(bp)                (ACT, PSUM->SBUF)
      Z   = sum_o e ; zinv = 1/Z   (DVE)
      m   = e * XH * zinv          (DVE, in-place)
      s  += sum_k m                (DVE reduce, then PE g-sum via S1)
    squash() is computed on [128, (o,b)] tiles with g-replicated layout.
"""

import sys

for _p in ("/opt/trn_rl_repo",):
    if _p not in sys.path:
        sys.path.insert(0, _p)

import numpy as np

B, I, DIN, O, DOUT = 256, 1152, 8, 10, 16
NCORES = 8
BL = B // NCORES          # 32 samples per core
G = 8                     # i's per contraction block
NK = I // G               # 144 k blocks
NCJ = 3                   # weight DMA chunks per o
KCJ = NK // NCJ           # 48 k per DMA chunk
KPS = 16                  # k per PSUM group in phase 1
KRC = 8                   # k per routing chunk
NRC = NK // KRC           # 18 routing chunks
EPS = 1e-8

_CACHE = {}


def _build_host_constants(weight):
    """Block-diagonal stationaries + selection matrices (host side)."""
    w5 = weight.reshape(O, NK, G, DOUT, DIN)          # [o,k,g,d,c]
    ws = np.zeros((O, NK, G, DIN, G, DOUT), np.float16)
    for g in range(G):
        # ws[o,k,g,c,g,d] = w5[o,k,g,d,c]
        ws[:, :, g, :, g, :] = np.swapaxes(w5[:, :, g, :, :], -1, -2)
    # [o, cj, r=64, kc=48, m=128]
    ws = ws.reshape(O, NK, G * DIN, G * DOUT)          # [o,k,64,128]
    ws = ws.reshape(O, NCJ, KCJ, 64, 128).transpose(0, 1, 3, 2, 4).copy()

    # selection matrices, layout p=(g,d) with p = g*16+d
    gi = np.arange(128) // DOUT   # g of row
    di = np.arange(128) % DOUT    # d of row
    s1 = (di[:, None] == di[None, :]).astype(np.float32)   # g-sum, replicated
    s2 = (gi[:, None] == gi[None, :]).astype(np.float32)   # d-sum, replicated
    s3 = np.full((128, 128), 0.125, np.float32)            # full sum / 8
    return ws, s1.astype(np.float32), s2.astype(np.float16), s3


def _build_program(routing_iters=2, phase1=True):
    import concourse.tile as tile
    from concourse import bacc, mybir

    f16 = mybir.dt.float16
    f32 = mybir.dt.float32
    AF = mybir.ActivationFunctionType
    AX = mybir.AxisListType

    nc = bacc.Bacc(
        "TRN2",
        target_bir_lowering=False,
        debug=False,
        enable_asserts=False,
        num_devices=NCORES,
    )

    ws_d = nc.dram_tensor("ws", [O, NCJ, 64, KCJ, 128], f16, kind="ExternalInput")
    xm_d = nc.dram_tensor("xm", [64, NK, BL], f16, kind="ExternalInput")
    s1_d = nc.dram_tensor("s1", [128, 128], f32, kind="ExternalInput")
    s2_d = nc.dram_tensor("s2", [128, 128], f16, kind="ExternalInput")
    s3_d = nc.dram_tensor("s3", [128, 128], f32, kind="ExternalInput")
    out_d = nc.dram_tensor("out", [DOUT, O, BL], f32, kind="ExternalOutput")

    with tile.TileContext(nc) as tc:
        with (
            tc.tile_pool(name="const", bufs=1) as const,
            tc.tile_pool(name="wpool", bufs=2) as wpool,
            tc.tile_pool(name="xhp", bufs=1) as xhp,
            tc.tile_pool(name="small", bufs=2) as small,
            tc.tile_pool(name="acc", bufs=1) as acc,
            tc.tile_pool(name="zp", bufs=2) as zp,
            tc.tile_pool(name="ep", bufs=2) as ep,
            tc.tile_pool(name="spsum", bufs=1, space="PSUM") as spsum,
        ):
            xm_sb = const.tile([64, NK, BL], f16)
            nc.gpsimd.dma_start(out=xm_sb[:], in_=xm_d.ap())
            s1_sb = const.tile([128, 128], f32)
            nc.gpsimd.dma_start(out=s1_sb[:], in_=s1_d.ap())
            s2_sb = const.tile([128, 128], f16)
            nc.gpsimd.dma_start(out=s2_sb[:], in_=s2_d.ap())
            s3_sb = const.tile([128, 128], f32)
            nc.gpsimd.dma_start(out=s3_sb[:], in_=s3_d.ap())

            xh = xhp.tile([128, O, NK, BL], f16)       # x_hat, p=(g,d)
            s0p = acc.tile([128, O, BL], f32)          # t=0 per-o k-sums

            # ---- Phase 1: x_hat = W @ x ------------------------------------
            with tc.tile_pool(name="ppsum", bufs=4, space="PSUM") as ppsum:
                for o in range(O):
                    for cj in range(NCJ if phase1 else 0):
                        wck = wpool.tile([64, KCJ, 128], f16)
                        nc.gpsimd.dma_start(out=wck[:], in_=ws_d.ap()[o, cj])
                        for pj in range(KCJ // KPS):
                            pt = ppsum.tile([128, KPS, BL], f32)
                            for kk in range(KPS):
                                k = cj * KCJ + pj * KPS + kk
                                nc.tensor.matmul(
                                    pt[:, kk, :],
                                    lhsT=wck[:, pj * KPS + kk, :],
                                    rhs=xm_sb[:, k, :],
                                    start=True,
                                    stop=True,
                                )
                            nc.scalar.copy(
                                out=xh[:, o, cj * KCJ + pj * KPS:
                                       cj * KCJ + (pj + 1) * KPS, :],
                                in_=pt[:],
                            )
                    # t=0 partial: sum over k (uniform routing weights)
                    nc.vector.reduce_sum(
                        out=s0p[:, o, :],
                        in_=xh[:, o, :, :].transpose([0, 2, 1]),
                        axis=AX.X,
                    )

            def squash(s_psum_ap, scale):
                """s_psum [128,(o,b)] fp32 g-replicated sums -> v [128,(o,b)] f32."""
                s_sb = small.tile([128, O, BL], f32, tag="sq_s")
                nc.scalar.mul(out=s_sb[:], in_=s_psum_ap, mul=scale)
                sq = small.tile([128, O, BL], f32, tag="sq_sq")
                nc.vector.tensor_mul(sq[:], s_sb[:], s_sb[:])
                m2p = spsum.tile([128, O, BL], f32, tag="sq_m2")
                nc.tensor.matmul(m2p[:], lhsT=s3_sb[:], rhs=sq[:],
                                 start=True, stop=True)
                rt = small.tile([128, O, BL], f32, tag="sq_rt")
                nc.scalar.sqrt(out=rt[:], in_=m2p[:])       # sqrt(mag2)
                nc.vector.tensor_scalar_add(rt[:], rt[:], EPS)
                den = small.tile([128, O, BL], f32, tag="sq_den")
                nc.scalar.add(out=den[:], in_=m2p[:], add=1.0)  # 1+mag2
                nc.vector.tensor_mul(den[:], den[:], rt[:])
                nc.vector.reciprocal_approx_fast(out=den[:], in_=den[:])
                fac = small.tile([128, O, BL], f32, tag="sq_fac")
                nc.vector.tensor_mul(fac[:], m2p[:], den[:])
                v = small.tile([128, O, BL], f32, tag="sq_v")
                nc.vector.tensor_mul(v[:], s_sb[:], fac[:])
                return v

            # ---- t = 0: uniform c = 1/10 -----------------------------------
            srp = spsum.tile([128, O, BL], f32, tag="srp")
            nc.tensor.matmul(srp[:], lhsT=s1_sb[:], rhs=s0p[:],
                             start=True, stop=True)
            v = squash(srp[:], 1.0 / O)
            vsum = acc.tile([128, O, BL], f32)
            nc.vector.tensor_copy(out=vsum[:], in_=v[:])
            vsum16 = acc.tile([128, O, BL], f16)
            nc.scalar.copy(out=vsum16[:], in_=vsum[:])

            # ---- t = 1, 2 ---------------------------------------------------
            sparts = acc.tile([128, NRC, O, BL], f32)
            with tc.tile_pool(name="bpsum", bufs=1, space="PSUM") as bpsum:
                for t in (1, 2)[:routing_iters]:
                    for kc in range(NRC):
                        ks = kc * KRC
                        # z = XH * vsum (broadcast over k)
                        z = zp.tile([128, O, KRC, BL], f16)
                        nc.vector.tensor_mul(
                            z[:],
                            xh[:, :, ks:ks + KRC, :],
                            vsum16[:].unsqueeze(2).broadcast_to(
                                (128, O, KRC, BL)),
                        )
                        # logits (g-grouped d-sums), replicated over d slots
                        bp = bpsum.tile([128, O, KRC, BL], f32)
                        for o in range(O):
                            nc.tensor.matmul(bp[:, o], lhsT=s2_sb[:],
                                             rhs=z[:, o], start=True, stop=True)
                        # e = exp(logits)
                        e = ep.tile([128, O, KRC, BL], f16)
                        nc.scalar.activation(out=e[:], in_=bp[:], func=AF.Exp)
                        # Z = sum_o e (binary tree keeps DVE in 2x mode)
                        t5 = small.tile([128, 5, KRC, BL], f16, tag="t5")
                        nc.vector.tensor_add(t5[:], e[:, 0:5], e[:, 5:10])
                        u2 = small.tile([128, 2, KRC, BL], f16, tag="u2")
                        nc.vector.tensor_add(u2[:], t5[:, 0:2], t5[:, 2:4])
                        w1 = small.tile([128, KRC, BL], f16, tag="w1")
                        nc.vector.tensor_add(w1[:], u2[:, 0], u2[:, 1])
                        zden = small.tile([128, KRC, BL], f32, tag="zden")
                        nc.vector.tensor_add(zden[:], w1[:], t5[:, 4])
                        # zinv = 1/Z (fast approx; Z in [~3.7, 27])
                        nc.vector.reciprocal_approx_fast(out=zden[:], in_=zden[:])
                        zinv16 = small.tile([128, KRC, BL], f16, tag="zinv")
                        nc.scalar.copy(out=zinv16[:], in_=zden[:])
                        # m = e * XH * zinv  (in place on e)
                        nc.vector.tensor_mul(e[:], e[:],
                                             xh[:, :, ks:ks + KRC, :])
                        nc.vector.tensor_mul(
                            e[:], e[:],
                            zinv16[:].unsqueeze(1).broadcast_to(
                                (128, O, KRC, BL)),
                        )
                        # s partial: sum over k in chunk (tree)
                        ka = small.tile([128, O, 4, BL], f16, tag="ka")
                        nc.vector.tensor_add(ka[:], e[:, :, 0:4], e[:, :, 4:8])
                        kb = small.tile([128, O, 2, BL], f16, tag="kb")
                        nc.vector.tensor_add(kb[:], ka[:, :, 0:2], ka[:, :, 2:4])
                        nc.vector.tensor_add(sparts[:, kc], kb[:, :, 0],
                                             kb[:, :, 1])
                    stot = small.tile([128, O, BL], f32, tag="stot")
                    nc.vector.reduce_sum(
                        out=stot[:], in_=sparts[:].transpose([0, 2, 3, 1]),
                        axis=AX.X)
                    srp2 = spsum.tile([128, O, BL], f32, tag="srp")
                    nc.tensor.matmul(srp2[:], lhsT=s1_sb[:], rhs=stot[:],
                                     start=True, stop=True)
                    v = squash(srp2[:], 1.0)
                    if t == 1:
                        nc.vector.tensor_add(vsum[:], vsum[:], v[:])
                        nc.scalar.copy(out=vsum16[:], in_=vsum[:])
                    else:
                        nc.gpsimd.dma_start(out=out_d.ap(), in_=v[0:DOUT])

    nc.compile()
    return nc


def _prepare_in_maps(inputs):
    x = np.asarray(inputs["x"], np.float32)
    weight = np.asarray(inputs["weight"], np.float32)
    ws, s1, s2, s3 = _build_host_constants(weight)

    # moving x blocks: xm[g*8+c, k, b] = x[b, k*8+g, c]
    x6 = x.reshape(B, NK, G, DIN)
    in_maps = []
    for core in range(NCORES):
        xl = x6[core * BL:(core + 1) * BL]                 # [b,k,g,c]
        xm = xl.transpose(2, 3, 1, 0).reshape(64, NK, BL)  # [(g,c),k,b]
        in_maps.append({
            "ws": ws,
            "xm": np.ascontiguousarray(xm, np.float16),
            "s1": s1,
            "s2": s2,
            "s3": s3,
        })
    return in_maps


def kernel(x, weight):
    from concourse.bass_utils import run_bass_kernel_spmd

    if "nc" not in _CACHE:
        _CACHE["nc"] = _build_program()
    nc = _CACHE["nc"]

    in_maps = _prepare_in_maps({"x": x, "weight": weight})

    res = run_bass_kernel_spmd(nc, in_maps, core_ids=list(range(NCORES)))
    _CACHE["last_results"] = res

    out = np.empty((B, O, DOUT), np.float32)
    for core in range(NCORES):
        oc = res.results[core]["out"]                      # [d, o, b]
        out[core * BL:(core + 1) * BL] = oc.transpose(2, 1, 0)
    return out



# revision 2
# speedup vs baseline: 1.2366x; 1.2366x over previous
"""DenseCapsule dynamic-routing kernel v2 for 8 Trainium2 NeuronCores.

Problem: x [256,1152,8] f32, weight [10,1152,16,8] f32 ->
  x_hat = einsum('oidc,bic->boid', weight, x)
  3 rounds of routing-by-agreement (softmax over o, squash over d)
  output [256, 10, 16] f32.

Layout (batch-parallel, 32 samples/core), i = kk*4 + g with g in [0,4):
  - Partition dim p = (b, g) = b*4+g (128 partitions).
  - Phase 1 matmul per kk (all at partition base 0, contraction 32):
      stationary XS[kk][(g,c), (b,g')] = delta_{g,g'} x[b, 4kk+g, c]
      moving     WV[kk][(g,c), (o,d)]  = W[o, 4kk+g, d, c]
    -> PSUM [(b,g), (o,d)] = x_hat[b, o, 4kk+g, d].
    Weight is NOT inflated (2.95 MB); x is inflated 4x (2.36 MB).
  - XH SBUF tile [128, KK, O, 8, 2] fp16 (d split 8x2 so the e-broadcast
    in the m-pass keeps unit innermost stride => DVE 2x mode).
  - t=0 capsule sums: PE-accumulated S1 matmuls over XH kk-triplets
    (S1 = delta_{b,b'} sums g within b, replicated over g').
  - Routing per t in kk-chunks: z = XH*vsum (1 TT), L = sum_d z
    (fp16 reduce), e = exp(L) on ACT engine written twice (pairs),
    Z = tree-sum over o, e' = e * (1/Z), m = XH*e' (per-o TT,
    pair-broadcast), s = S1-supers over m (480-col moving, LDW hidden).
  - squash entirely on [*, O, 8, 2] tiles (d in free dim, no PE).
"""

import sys

for _p in ("/opt/trn_rl_repo",):
    if _p not in sys.path:
        sys.path.insert(0, _p)

import numpy as np

B, I, DIN, O, DOUT = 256, 1152, 8, 10, 16
NCORES = 8
BL = B // NCORES          # 32 samples per core
G = 4                     # i's per phase-1 contraction block
KK = I // G               # 288 kk blocks
KC = 96                   # kk per routing chunk
NJ = (I * DIN) // 128     # 72 s0 chunks of 128 (i,c) rows
NCH = KK // KC            # routing chunks per iteration
TR = 3                    # kk per s-reduce matmul (3*160=480 <= 512)
ND = 12                   # input DMA chunks
KD = KK // ND             # kk per DMA chunk
EPS = 1e-8

_CACHE = {}


def _build_host_constants(weight):
    w5 = weight.reshape(O, KK, G, DOUT, DIN)           # [o,kk,g,d,c]
    wkgc = w5.transpose(1, 2, 4, 0, 3)                 # [kk,g,c,o,d]
    # wv[(g,c), kk, (o,d)] -> [32, KK, 160]
    wv = np.ascontiguousarray(
        wkgc.reshape(KK, 32, O * DOUT).transpose(1, 0, 2)).astype(np.float16)
    # wvj[(i,c) % 128, j, (o,d)] -> [128, NJ, 160]
    wvj = np.ascontiguousarray(
        wkgc.reshape(NJ, 128, O * DOUT).transpose(1, 0, 2)).astype(np.float16)

    bi = np.arange(128) // G
    s1 = (bi[:, None] == bi[None, :]).astype(np.float16)     # [128,128]
    s1f = (bi[:, None] == np.arange(BL)[None, :]).astype(np.float16)
    return wv, wvj, s1, s1f


def _per_core_inputs(xl, wv, wvj, s1, s1f):
    """xl: [BL, I, DIN] fp32 slice for this core."""
    x4 = xl.reshape(BL, KK, G, DIN).astype(np.float16)  # [b,kk,g,c]
    # xs[kk, (g,c), (b,g')] = delta_{g,g'} x[b, 4kk+g, c]
    xs = np.zeros((KK, G, DIN, BL, G), np.float16)      # [kk,g,c,b,g']
    xkcb = x4.transpose(1, 2, 3, 0)                     # [kk,g,c,b]
    for g in range(G):
        xs[:, g, :, :, g] = xkcb[:, g, :, :]
    xs = np.ascontiguousarray(
        xs.reshape(KK, 32, 128).transpose(1, 0, 2))     # [32, KK, 128]

    # xt4[(i,c), (b,g)] = x[b, i, c]
    xt = xl.reshape(BL, I * DIN).T.astype(np.float16)   # [(i,c), b]
    xt4 = np.repeat(xt, G, axis=1)                      # [(i,c), (b,g)]
    xt4 = np.ascontiguousarray(
        xt4.reshape(NJ, 128, 128).transpose(1, 0, 2))   # [128, NJ, 128]
    return {"xs": xs, "wv": wv, "wvj": wvj, "s1": s1, "s1f": s1f,
            "xt4": xt4}


def _squash(nc, small, s_ap, scale, f32, AX, ALU, NP):
    """squash(s*scale) on [NP, O, 8, 2] fp32; returns fp32 tile."""
    s_sb = small.tile([NP, O, 8, 2], f32, tag=f"sq_s{NP}")
    nc.scalar.mul(out=s_sb[:], in_=s_ap, mul=float(scale))
    sq = small.tile([NP, O, 8, 2], f32, tag=f"sq_sq{NP}")
    nc.vector.tensor_mul(sq[:], s_sb[:], s_sb[:])
    m2 = small.tile([NP, O, 1, 1], f32, tag=f"sq_m2{NP}")
    nc.vector.tensor_reduce(out=m2[:], in_=sq[:], axis=AX.XY, op=ALU.add)
    rt = small.tile([NP, O, 1, 1], f32, tag=f"sq_rt{NP}")
    nc.scalar.sqrt(out=rt[:], in_=m2[:])            # sqrt(mag2)
    nc.vector.tensor_scalar_add(rt[:], rt[:], EPS)
    den = small.tile([NP, O, 1, 1], f32, tag=f"sq_den{NP}")
    nc.scalar.add(out=den[:], in_=m2[:], add=1.0)   # 1 + mag2
    nc.vector.tensor_mul(den[:], den[:], rt[:])
    nc.vector.reciprocal_approx_fast(out=den[:, :, 0, 0], in_=den[:, :, 0, 0])
    fac = small.tile([NP, O, 1, 1], f32, tag=f"sq_fac{NP}")
    nc.vector.tensor_mul(fac[:], m2[:], den[:])
    v = small.tile([NP, O, 8, 2], f32, tag=f"sq_v{NP}")
    nc.vector.tensor_mul(v[:], s_sb[:], fac[:].broadcast_to((NP, O, 8, 2)))
    return v


def _build_program():
    import concourse.tile as tile
    from concourse import bacc, mybir

    f16 = mybir.dt.float16
    f32 = mybir.dt.float32
    AF = mybir.ActivationFunctionType
    AX = mybir.AxisListType
    ALU = mybir.AluOpType

    nc = bacc.Bacc(
        "TRN2",
        target_bir_lowering=False,
        debug=False,
        enable_asserts=False,
        num_devices=NCORES,
    )

    xs_d = nc.dram_tensor("xs", [32, KK, 128], f16, kind="ExternalInput")
    wv_d = nc.dram_tensor("wv", [32, KK, O * DOUT], f16, kind="ExternalInput")
    wvj_d = nc.dram_tensor("wvj", [128, NJ, O * DOUT], f16, kind="ExternalInput")
    xt4_d = nc.dram_tensor("xt4", [128, NJ, 128], f16, kind="ExternalInput")
    s1_d = nc.dram_tensor("s1", [128, 128], f16, kind="ExternalInput")
    s1f_d = nc.dram_tensor("s1f", [128, BL], f16, kind="ExternalInput")
    out_d = nc.dram_tensor("out", [BL, O, 8, 2], f32, kind="ExternalOutput")

    with tile.TileContext(nc) as tc:
        with (
            tc.tile_pool(name="const", bufs=1) as const,
            tc.tile_pool(name="xhp", bufs=1) as xhp,
            tc.tile_pool(name="acc", bufs=1) as acc,
            tc.tile_pool(name="small", bufs=1) as small,
            tc.tile_pool(name="zmp", bufs=2) as zmp,
            tc.tile_pool(name="spsum", bufs=1, space="PSUM") as spsum,
        ):
            s1_sb = const.tile([128, 128], f16)
            nc.gpsimd.dma_start(out=s1_sb[:], in_=s1_d.ap())
            s1f_sb = const.tile([128, BL], f16)
            nc.gpsimd.dma_start(out=s1f_sb[:], in_=s1f_d.ap())

            # x_hat, p=(b,g), free (kk, o, dd, r) with d = dd*2+r
            xh = xhp.tile([128, KK, O, 8, 2], f16)
            vsumh = acc.tile([128, O, 8, 2], f16)
            vsum = acc.tile([128, O, 8, 2], f32)

            # ---- Phase 1: x_hat + direct t=0 sums -----------------------
            JD = NJ // ND
            s0sp = spsum.tile([128, O, 8, 2], f32, tag="sp128")
            with (
                tc.tile_pool(name="wpool", bufs=2) as wpool,
                tc.tile_pool(name="xspool", bufs=2) as xspool,
                tc.tile_pool(name="wjpool", bufs=2) as wjpool,
                tc.tile_pool(name="xtpool", bufs=2) as xtpool,
                tc.tile_pool(name="ppsum", bufs=4, space="PSUM") as ppsum,
            ):
                NDS = 4                # dc's carrying the s0 work
                JD2 = NJ // NDS
                for dc in range(ND):
                    wck = wpool.tile([32, KD, O * DOUT], f16)
                    nc.gpsimd.dma_start(
                        out=wck[:], in_=wv_d.ap()[:, dc * KD:(dc + 1) * KD])
                    xsk = xspool.tile([32, KD, 128], f16)
                    nc.gpsimd.dma_start(
                        out=xsk[:], in_=xs_d.ap()[:, dc * KD:(dc + 1) * KD])
                    if dc < NDS:
                        for sub in range(JD2 // JD):
                            j0 = dc * JD2 + sub * JD
                            wjk = wjpool.tile([128, JD, O * DOUT], f16)
                            nc.gpsimd.dma_start(
                                out=wjk[:], in_=wvj_d.ap()[:, j0:j0 + JD])
                            xtk = xtpool.tile([128, JD, 128], f16)
                            nc.gpsimd.dma_start(
                                out=xtk[:], in_=xt4_d.ap()[:, j0:j0 + JD])
                            for jj in range(JD):
                                j = j0 + jj
                                # s0 = sum_(i,c) x*W : accumulate over j
                                nc.tensor.matmul(
                                    s0sp[:],
                                    lhsT=xtk[:, jj, :],
                                    rhs=wjk[:, jj, :],
                                    start=(j == 0),
                                    stop=(j == NJ - 1),
                                )
                    for s in range(KD // TR):
                        pt = ppsum.tile([128, TR, O, 8, 2], f32)
                        for r in range(TR):
                            nc.tensor.matmul(
                                pt[:, r],
                                lhsT=xsk[:, s * TR + r, :],
                                rhs=wck[:, s * TR + r, :],
                                start=True,
                                stop=True,
                            )
                        kk0 = dc * KD + s * TR
                        dst = xh[:, kk0:kk0 + TR]
                        if s % 6 == 0:
                            nc.vector.tensor_copy(out=dst, in_=pt[:])
                        else:
                            nc.scalar.copy(out=dst, in_=pt[:])
                    if dc == NDS - 1:
                        # ---- t = 0: uniform c = 1/10 (early) ------------
                        v = _squash(nc, small, s0sp[:], 1.0 / O,
                                    f32, AX, ALU, 128)
                        nc.vector.tensor_copy(out=vsum[:], in_=v[:])
                        nc.scalar.copy(out=vsumh[:], in_=vsum[:])

            # ---- t = 1, 2 ------------------------------------------------
            with nc.allow_low_precision(reason="logits/softmax in fp16"):
                for t in (1, 2):
                    final = t == 2
                    sS = s1f_sb if final else s1_sb
                    NP = BL if final else 128
                    sp = spsum.tile([NP, TR, O, 8, 2], f32, tag=f"tsp{NP}")
                    for ch in range(NCH):
                        k0 = ch * KC
                        zm = zmp.tile([128, KC, O, 8, 2], f16, tag="zm")
                        # z = XH * vsum  (vsum bcast over kk)
                        nc.vector.tensor_mul(
                            zm[:],
                            xh[:, k0:k0 + KC],
                            vsumh[:].unsqueeze(1)
                            .broadcast_to((128, KC, O, 8, 2)),
                        )
                        # L = sum_d z -> [128, KC, O] fp16 (in-place pair
                        # tree in zm; all levels unit-stride => DVE 2x)
                        nc.vector.tensor_add(
                            zm[:, :, :, 0:4], zm[:, :, :, 0:4], zm[:, :, :, 4:8])
                        nc.vector.tensor_add(
                            zm[:, :, :, 0:2], zm[:, :, :, 0:2], zm[:, :, :, 2:4])
                        nc.vector.tensor_add(
                            zm[:, :, :, 0:1], zm[:, :, :, 0:1], zm[:, :, :, 1:2])
                        L = small.tile([128, KC, O], f16, tag="L")
                        nc.vector.tensor_add(
                            L[:], zm[:, :, :, 0, 0], zm[:, :, :, 0, 1])
                        # e = exp(L), written twice (pairs)
                        e2 = small.tile([128, KC, O, 2], f16, tag="e2")
                        nc.scalar.activation(
                            out=e2[:, :, :, 0], in_=L[:], func=AF.Exp)
                        nc.scalar.activation(
                            out=e2[:, :, :, 1], in_=L[:], func=AF.Exp)
                        # Z = sum_o e (pairs tree)
                        t5 = small.tile([128, KC, 5, 2], f16, tag="t5")
                        nc.vector.tensor_add(t5[:], e2[:, :, 0:5], e2[:, :, 5:10])
                        u2 = small.tile([128, KC, 2, 2], f16, tag="u2")
                        nc.vector.tensor_add(u2[:], t5[:, :, 0:2], t5[:, :, 2:4])
                        zden = small.tile([128, KC, 1, 2], f32, tag="zden")
                        nc.vector.tensor_add(zden[:], u2[:, :, 0:1], u2[:, :, 1:2])
                        nc.vector.tensor_add(zden[:], zden[:], t5[:, :, 4:5])
                        nc.vector.reciprocal_approx_fast(
                            out=zden[:, :, 0, :], in_=zden[:, :, 0, :])
                        zinv = small.tile([128, KC, 1, 2], f16, tag="zinv")
                        nc.vector.tensor_copy(out=zinv[:], in_=zden[:])
                        # e' = e * (1/Z)  (bcast over o)
                        nc.vector.tensor_mul(
                            e2[:], e2[:], zinv[:].broadcast_to((128, KC, O, 2)))
                        # m = XH * e'  (pair-bcast over dd) -- per o
                        for o in range(O):
                            nc.vector.tensor_mul(
                                zm[:, :, o],
                                xh[:, k0:k0 + KC, o],
                                e2[:, :, o].unsqueeze(2)
                                .broadcast_to((128, KC, 8, 2)),
                            )
                        # s += sum_{kk,g} m : PE accumulation, kk-triplets
                        for s in range(KC // TR):
                            nc.tensor.matmul(
                                sp[:],
                                lhsT=sS[:],
                                rhs=zm[:, TR * s:TR * s + TR],
                                start=(ch == 0 and s == 0),
                                stop=(ch == NCH - 1 and s == KC // TR - 1),
                            )
                    stot = small.tile([NP, O, 8, 2], f32, tag=f"stot{NP}")
                    nc.scalar.copy(out=stot[:], in_=sp[:, 0])
                    nc.vector.tensor_add(stot[:], stot[:], sp[:, 1])
                    nc.vector.tensor_add(stot[:], stot[:], sp[:, 2])
                    v = _squash(nc, small, stot[:], 1.0, f32, AX, ALU, NP)
                    if final:
                        nc.gpsimd.dma_start(out=out_d.ap(), in_=v[:])
                    else:
                        nc.vector.tensor_add(vsum[:], vsum[:], v[:])
                        nc.scalar.copy(out=vsumh[:], in_=vsum[:])

    nc.compile()
    return nc


def _prepare_in_maps(inputs):
    x = np.asarray(inputs["x"], np.float32)
    weight = np.asarray(inputs["weight"], np.float32)
    wv, wvj, s1, s1f = _build_host_constants(weight)
    in_maps = []
    for core in range(NCORES):
        xl = x[core * BL:(core + 1) * BL]
        in_maps.append(_per_core_inputs(xl, wv, wvj, s1, s1f))
    return in_maps


def kernel(x, weight):
    from concourse.bass_utils import run_bass_kernel_spmd

    if "nc" not in _CACHE:
        _CACHE["nc"] = _build_program()
    nc = _CACHE["nc"]

    in_maps = _prepare_in_maps({"x": x, "weight": weight})
    res = run_bass_kernel_spmd(nc, in_maps, core_ids=list(range(NCORES)))
    _CACHE["last_results"] = res

    out = np.empty((B, O, DOUT), np.float32)
    for core in range(NCORES):
        oc = res.results[core]["out"]              # [BL, O, 8, 2]
        out[core * BL:(core + 1) * BL] = oc.reshape(BL, O, DOUT)
    return out
